# revision 10
# baseline (speedup 1.0000x reference)
"""Single-head attention (B=4, T=4096, E=1024, H=64) on 8 trn2 NeuronCores.

Sharding: 2 cores per batch element; each core computes the full K/V
projections for its batch element but only its half of the queries
(sequence-parallel over queries, data-parallel over batch). The host
permutes each core's token order so its own query half comes first —
attention is permutation-invariant over keys, so every core runs an
identical SPMD program with no collectives.

Per-core on-chip pipeline (all layouts transposed, feature-on-partition):
  x is host-swizzled tile-contiguous and fully prefetched to SBUF in
  8 x 1MB dense DMAs (decouples DMA from compute pacing).
  xT [E,T] bf16 --matmul (Wk|Wv) packed--> K^T,V^T [64,T] f32
             --matmul Wq (first T/2 cols)--> Q^T [64,T/2]
  V^T --DMA-xbar transpose (Activation HWDGE queue)--> Vst [T/128,128,64|ones]
  S^T chunk = K^T_chunk.T @ Q^T  (contraction H=64, rowtiled, PSUM [128,1024])
  P^T = exp(S^T/8): split across two engines per a static pattern --
        ScalarE table exp, or VectorE Schraudolph exp (tensor_scalar
        f32*A+B -> int16 bits reinterpreted as bf16; ~1.8% rms)
  O^T += [V|ones].T @ P^T  (PSUM [128,512]: rows 0:64 = O^T,
                            rows 64:128 = softmax denominator)
  out = O^T * reciprocal_approx_fast(l)  (VectorE), DMA out [64, T/2] f32
"""

import os
import sys

import numpy as np

E, T, H, B = 1024, 4096, 64, 4
NCORES = 8
TQ = T // 2

_BUILT = {}
LAST_RESULT = None  # stashed BassKernelResults for test harness introspection

# Schraudolph bf16 exp: bits_i16 = floor(x*SCHRD_A + SCHRD_B); the +0.5 in B
# converts the observed floor conversion into round-half-up. C=7.5 centers
# the relative error (~1.8% rms, mean ~0).
SCHRD_C = 7.5
# number of groups (of 8) routed to the DVE Schraudolph exp; rest -> ScalarE
DVE_SLOTS = frozenset((2, 5, 7))


def _ensure_paths():
    for p in ("/opt/trn_rl_repo",):
        if p not in sys.path:
            sys.path.insert(0, p)


def _legalize_waits(nc, mybir, max_waits=1):
    """This walrus build only accepts 1 sem-wait per instruction; Tile's
    tail drains carry several. Move excess waits onto injected NoOps on
    the same engine right before the offending instruction."""
    ctr = 0
    for bb in nc.main_func.blocks:
        new_list = []
        for ins in bb.instructions:
            si = ins.sync_info
            if si is not None and len(si.on_wait) > max_waits:
                waits = list(si.on_wait)
                extra, keep = waits[:-max_waits], waits[-max_waits:]
                while extra:
                    chunk, extra = extra[:max_waits], extra[max_waits:]
                    ctr += 1
                    nop = mybir.InstNoOp(name=f"WFIX-{id(nc) & 0xFFFF}-{ctr}")
                    nop.engine = ins.engine
                    nop.sync_info = mybir.SyncInfo(on_wait=chunk, on_update=[])
                    new_list.append(nop)
                ins.sync_info = mybir.SyncInfo(
                    on_wait=keep, on_update=list(si.on_update)
                )
            new_list.append(ins)
        bb.instructions[:] = new_list


def _install_ntff_hook():
    """The image's antenv lacks axon_hooks, so trace=True degrades. Inject
    the module backed by the boot helper's ctypes implementation."""
    import types

    if "antenv.axon_hooks" in sys.modules:
        return
    if "/root/.axon_site" not in sys.path:
        sys.path.insert(0, "/root/.axon_site")
    try:
        from trn_agent_boot.trn_boot import _ntff_profile_via_ctypes

        hook = _ntff_profile_via_ctypes("/opt/axon/libaxon_pjrt.so")
    except Exception:
        return
    mod = types.ModuleType("antenv.axon_hooks")
    mod.get_axon_ntff_profile_hook = lambda: hook
    mod.set_axon_ntff_profile_hook = lambda h: None
    sys.modules["antenv.axon_hooks"] = mod


def build_nc(
    e=E,
    t=T,
    tq=TQ,
    gfd=1024,
    dve_slots=DVE_SLOTS,
    legalize=True,
    use_dma_transpose=True,
    use_scalar_dup=True,
    use_gpsimd_memset=True,
    use_recip_approx=True,
):
    """Emit the SPMD per-core program. Shapes parameterized so the same
    builder is validated in CoreSim at mini scale."""
    _ensure_paths()
    import concourse.bass as bass
    import concourse.mybir as mybir
    import concourse.tile as tile
    from concourse.masks import make_identity
    from contextlib import ExitStack

    f32 = mybir.dt.float32
    bf16 = mybir.dt.bfloat16
    i16 = mybir.dt.int16
    Exp = mybir.ActivationFunctionType.Exp
    Log = mybir.ActivationFunctionType.Ln

    EC = e // 128      # E (contraction) chunks for projections
    TT = t // 512      # token tiles (projection streaming)
    TTQ = tq // 512    # token tiles that also need Q projection
    KC = t // 128      # key chunks (attention contraction)
    QTN = tq // 512    # query tiles in attention
    GK = gfd // 512    # key chunks per exp group
    NG = KC // GK      # exp groups per query tile
    assert KC % GK == 0 and GK == 2

    scale = 1.0 / float(np.sqrt(H))
    schrd_a = float(128.0 / np.log(2.0)) * scale
    schrd_b = float(127 * 128 - SCHRD_C + 0.5)

    nc = bass.Bass()
    # host-swizzled x: [n, p, c, u] so each token tile is one dense 2D DMA
    xT4 = nc.declare_dram_parameter("xT4", [TT, 128, EC, 512], bf16, False)
    wkv = nc.declare_dram_parameter("wkv", [e, 2 * H], bf16, False)
    wq = nc.declare_dram_parameter("wq", [e, H], bf16, False)
    outT = nc.declare_dram_parameter("outT", [H, tq], f32, True)

    wkv_r = wkv.rearrange("(c p) m -> p c m", p=128)
    wq_r = wq.rearrange("(c p) m -> p c m", p=128)

    with ExitStack() as ctx:
        tc = ctx.enter_context(tile.TileContext(nc))
        singles = ctx.enter_context(tc.tile_pool(name="singles", bufs=1))
        ppool = ctx.enter_context(tc.tile_pool(name="ppool", bufs=3))
        rpool = ctx.enter_context(tc.tile_pool(name="rpool", bufs=2))
        spool = ctx.enter_context(tc.tile_pool(name="spool", bufs=3, space="PSUM"))
        opool = ctx.enter_context(tc.tile_pool(name="opool", bufs=2, space="PSUM"))

        wkv_sb = singles.tile([128, EC, 2 * H], bf16)
        nc.sync.dma_start(out=wkv_sb, in_=wkv_r)
        wq_sb = singles.tile([128, EC, H], bf16)
        nc.sync.dma_start(out=wq_sb, in_=wq_r)

        # full x prefetch: one dense [128, EC*512] DMA per token tile
        xt = [singles.tile([128, EC, 512], bf16, name=f"x{n}") for n in range(TT)]
        for n in range(TT):
            nc.sync.dma_start(out=xt[n], in_=xT4[n])

        # K^T rowtiled: [128, 256] per token tile -- even key chunks on
        # partitions 0:64, odd on 64:128, two 128-col blocks per tile.
        KTp = [singles.tile([128, 256], bf16, name=f"KT{n}") for n in range(TT)]
        # Q^T per query tile, duplicated across both partition halves (each
        # concurrent row-tile streams rhs from its own partition range).
        QTp = [singles.tile([128, 512], bf16, name=f"QT{n}") for n in range(TTQ)]
        VTtp = [singles.tile([H, 512], bf16, name=f"VTt{n}") for n in range(TT)]
        # PV stationary: [V_chunk | ones] per key chunk, ones replicated to
        # fill M=128 so rows 64:128 of the PV accumulator hold the softmax
        # denominator. One big tile; V written by DMA-xbar transpose.
        Vst3 = singles.tile([128, KC, 128], bf16)
        if use_gpsimd_memset:
            nc.gpsimd.memset(Vst3[:, :, H:], 1.0)
        else:
            nc.vector.memset(Vst3[:, :, H:], 1.0)
        OTp = [singles.tile([H, 512], f32, name=f"OT{q}") for q in range(QTN)]
        if not use_dma_transpose:
            ident = singles.tile([H, H], bf16)
            make_identity(nc, ident)

        o_ps_list = [None] * QTN
        gctr = [0]  # global group counter for the exp-engine pattern

        def emit_group(q, g, o_ps):
            s_ps = spool.tile([128, gfd], f32, tag="s", name=f"s{q}_{g}")
            kt = KTp[g // 2][:, (g % 2) * 128:(g % 2 + 1) * 128]
            nc.tensor.matmul(
                s_ps[:, 0:512], kt[0:H], QTp[q][0:H, :],
                start=True, stop=True, skip_group_check=True,
            )
            nc.tensor.matmul(
                s_ps[:, 512:1024], kt[H:128], QTp[q][H:128, :],
                start=True, stop=True, skip_group_check=True,
                tile_position=(64, 0),
            )
            pt = ppool.tile([128, gfd], bf16, tag="p", name=f"p{q}_{g}")
            if (gctr[0] % 8) in dve_slots:
                nc.vector.tensor_scalar(
                    out=pt[:].bitcast(i16),
                    in0=s_ps[:],
                    scalar1=schrd_a,
                    scalar2=schrd_b,
                    op0=mybir.AluOpType.mult,
                    op1=mybir.AluOpType.add,
                )
            else:
                nc.scalar.activation(pt, s_ps, Exp, scale=scale)
            gctr[0] += 1
            for k in range(GK):
                c = g * GK + k
                nc.tensor.matmul(
                    o_ps, Vst3[:, c, :], pt[:, k * 512:(k + 1) * 512],
                    start=(c == 0), stop=(c == KC - 1),
                    skip_group_check=True,
                )

        def emit_finalize(q, o_ps):
            rec = rpool.tile([H, 512], f32, tag="rec", name=f"rec{q}")
            if use_recip_approx:
                # 1/l = exp(-ln l) on ScalarE; ln+exp share one table set
                # (natural_log_exp_and_others) so no mid-kernel reload.
                lnl = rpool.tile([H, 512], f32, tag="lnl", name=f"lnl{q}")
                nc.scalar.activation(lnl, o_ps[H:128, :], Log)
                nc.scalar.activation(rec, lnl, Exp, scale=-1.0)
            else:
                nc.vector.reciprocal(rec, o_ps[H:128, :])
            nc.vector.tensor_mul(OTp[q][:], o_ps[0:H, :], rec)
            nc.sync.dma_start(
                out=outT[:, q * 512:(q + 1) * 512], in_=OTp[q][:]
            )

        # ---- emission: projections interleaved with q0 attention ----
        for n in range(TT):
            kv_ps = spool.tile([128, 512], f32, tag="s", name=f"kv{n}")
            for c in range(EC):
                nc.tensor.matmul(
                    kv_ps, wkv_sb[:, c, :], xt[n][:, c, :],
                    start=(c == 0), stop=(c == EC - 1),
                )
            srcv = kv_ps[0:H, :].rearrange("h (i r u) -> h i r u", i=2, r=2, u=128)
            dst = KTp[n].rearrange("p (i u) -> p i u", u=128)
            nc.vector.tensor_copy(dst[0:H], srcv[:, :, 0, :])
            nc.vector.tensor_copy(dst[H:128], srcv[:, :, 1, :])
            nc.vector.tensor_copy(VTtp[n][:], kv_ps[H:128, :])
            # V chunks -> [keys, dims] via DMA-xbar transpose on the
            # Activation HWDGE queue (doesn't queue behind the x prefetch)
            for j in range(4):
                c = 4 * n + j
                if use_dma_transpose:
                    nc.scalar.dma_start_transpose(
                        out=Vst3[:, c, 0:H],
                        in_=VTtp[n][:, j * 128:(j + 1) * 128],
                    )
                else:
                    tp = opool.tile([128, H], bf16, tag="o", name=f"tp{c}")
                    nc.tensor.transpose(
                        tp, VTtp[n][:, j * 128:(j + 1) * 128], ident
                    )
                    nc.vector.tensor_copy(Vst3[:, c, 0:H], tp)
            if n < TTQ:
                q_ps = spool.tile([H, 512], f32, tag="s", name=f"q{n}")
                for c in range(EC):
                    nc.tensor.matmul(
                        q_ps, wq_sb[:, c, :], xt[n][:, c, :],
                        start=(c == 0), stop=(c == EC - 1),
                    )
                nc.vector.tensor_copy(QTp[n][0:H, :], q_ps)
                if use_scalar_dup:
                    nc.scalar.dma_start(out=QTp[n][H:128, :], in_=QTp[n][0:H, :])
                else:
                    nc.vector.tensor_copy(QTp[n][H:128, :], q_ps)
            # q0 attention groups feasible with tiles <= n
            if o_ps_list[0] is None:
                o_ps_list[0] = opool.tile([128, 512], f32, tag="o", name="o0")
            for g in range(2 * n, min(2 * n + 2, NG)):
                emit_group(0, g, o_ps_list[0])

        for g in range(2 * TT, NG):
            emit_group(0, g, o_ps_list[0])
        emit_finalize(0, o_ps_list[0])
        for q in range(1, QTN):
            o_ps_list[q] = opool.tile([128, 512], f32, tag="o", name=f"o{q}")
            for g in range(NG):
                emit_group(q, g, o_ps_list[q])
            emit_finalize(q, o_ps_list[q])

    if legalize:
        _legalize_waits(nc, __import__("concourse.mybir", fromlist=["x"]))
    return nc


def _get_nc():
    key = (E, T, TQ)
    if key not in _BUILT:
        _BUILT[key] = build_nc()
    return _BUILT[key]


def _swizzle_xT(xT):
    """[E, T] f32 -> [TT, 128, EC, 512] bf16 tile-contiguous layout."""
    import ml_dtypes

    e, t = xT.shape
    ec, tt = e // 128, t // 512
    v = xT.reshape(ec, 128, tt, 512).transpose(2, 1, 0, 3)
    return np.ascontiguousarray(v).astype(ml_dtypes.bfloat16)


def kernel(x, Wq, Wk, Wv):
    """Full inputs -> full output, distributing over 8 NeuronCores."""
    _ensure_paths()
    _install_ntff_hook()
    import ml_dtypes
    from concourse.bass_utils import run_bass_kernel_spmd

    global LAST_RESULT

    nc = _get_nc()

    x = np.asarray(x, np.float32)
    wkv_np = np.ascontiguousarray(
        np.concatenate([np.asarray(Wk, np.float32), np.asarray(Wv, np.float32)], axis=1)
    ).astype(ml_dtypes.bfloat16)
    wq_np = np.ascontiguousarray(np.asarray(Wq, np.float32)).astype(ml_dtypes.bfloat16)

    in_maps = []
    for core in range(NCORES):
        b, half = divmod(core, 2)
        o = TQ if half == 0 else 0
        idx = np.r_[half * TQ:(half + 1) * TQ, o:o + TQ]
        xT4 = _swizzle_xT(np.ascontiguousarray(x[b, idx].T))
        in_maps.append({"xT4": xT4, "wkv": wkv_np, "wq": wq_np})

    trace = bool(os.environ.get("KERNEL_TRACE"))
    res = run_bass_kernel_spmd(nc, in_maps, list(range(NCORES)), trace=trace)
    LAST_RESULT = res

    out = np.empty((B, T, H), np.float32)
    for core in range(NCORES):
        b, half = divmod(core, 2)
        out[b, half * TQ:(half + 1) * TQ, :] = res.results[core]["outT"].T
    return out


# revision 17
# speedup vs baseline: 1.2118x; 1.2118x over previous
"""Single-head attention (B=4, T=4096, E=1024, H=64) on 8 trn2 NeuronCores.

Sharding: 2 cores per batch element; each core computes the full K/V
projections for its batch element but only its half of the queries
(sequence-parallel over queries, data-parallel over batch). The host
permutes each core's token order so its own query half comes first —
attention is permutation-invariant over keys, so every core runs an
identical SPMD program with no collectives.

Per-core on-chip pipeline (all layouts transposed, feature-on-partition):
  x is host-swizzled tile-contiguous and fully prefetched to SBUF in
  8 x 1MB dense DMAs (decouples DMA from compute pacing).
  xT [E,T] bf16 --matmul (Wk|Wv) packed--> K^T,V^T [64,T] f32
             --matmul Wq (first T/2 cols)--> Q^T [64,T/2]
  V^T --DMA-xbar transpose (Activation HWDGE queue)--> Vst [T/128,128,64|ones]
  S^T chunk = K^T_chunk.T @ Q^T  (contraction H=64, rowtiled, PSUM [128,1024])
  P^T = exp(S^T/8): split across two engines per a static pattern --
        ScalarE table exp, or VectorE Schraudolph exp (tensor_scalar
        f32*A+B -> int16 bits reinterpreted as bf16; ~1.8% rms)
  O^T += [V|ones].T @ P^T  (PSUM [128,512]: rows 0:64 = O^T,
                            rows 64:128 = softmax denominator)
  out = O^T * exp(-ln l)  (ScalarE recip, VectorE mul), DMA out [64, T/2] f32
"""

import os
import sys

import numpy as np

E, T, H, B = 1024, 4096, 64, 4
NCORES = 8
TQ = T // 2

_BUILT = {}
LAST_RESULT = None  # stashed BassKernelResults for test harness introspection

# Schraudolph bf16 exp: bits_i16 = floor(x*SCHRD_A + SCHRD_B); the +0.5 in B
# converts the observed floor conversion into round-half-up. C=7.5 centers
# the relative error (~1.8% rms, mean ~0).
SCHRD_C = 7.5
# number of groups (of 8) routed to the DVE Schraudolph exp; rest -> ScalarE
DVE_SLOTS = frozenset((2, 5, 7))


def _ensure_paths():
    for p in ("/opt/trn_rl_repo",):
        if p not in sys.path:
            sys.path.insert(0, p)


def _legalize_waits(nc, mybir, max_waits=1):
    """This walrus build only accepts 1 sem-wait per instruction; Tile's
    tail drains carry several. Move excess waits onto injected NoOps on
    the same engine right before the offending instruction."""
    ctr = 0
    for bb in nc.main_func.blocks:
        new_list = []
        for ins in bb.instructions:
            si = ins.sync_info
            if si is not None and len(si.on_wait) > max_waits:
                waits = list(si.on_wait)
                extra, keep = waits[:-max_waits], waits[-max_waits:]
                while extra:
                    chunk, extra = extra[:max_waits], extra[max_waits:]
                    ctr += 1
                    nop = mybir.InstNoOp(name=f"WFIX-{id(nc) & 0xFFFF}-{ctr}")
                    nop.engine = ins.engine
                    nop.sync_info = mybir.SyncInfo(on_wait=chunk, on_update=[])
                    new_list.append(nop)
                ins.sync_info = mybir.SyncInfo(
                    on_wait=keep, on_update=list(si.on_update)
                )
            new_list.append(ins)
        bb.instructions[:] = new_list


def _install_ntff_hook():
    """The image's antenv lacks axon_hooks, so trace=True degrades. Inject
    the module backed by the boot helper's ctypes implementation."""
    import types

    if "antenv.axon_hooks" in sys.modules:
        return
    if "/root/.axon_site" not in sys.path:
        sys.path.insert(0, "/root/.axon_site")
    try:
        from trn_agent_boot.trn_boot import _ntff_profile_via_ctypes

        hook = _ntff_profile_via_ctypes("/opt/axon/libaxon_pjrt.so")
    except Exception:
        return
    mod = types.ModuleType("antenv.axon_hooks")
    mod.get_axon_ntff_profile_hook = lambda: hook
    mod.set_axon_ntff_profile_hook = lambda h: None
    sys.modules["antenv.axon_hooks"] = mod


def build_nc(
    e=E,
    t=T,
    tq=TQ,
    gfd=1024,
    dve_slots=DVE_SLOTS,
    legalize=True,
    use_dma_transpose=False,
    use_scalar_dup=True,
    use_gpsimd_memset=True,
    use_recip_approx=True,
):
    """Emit the SPMD per-core program. Shapes parameterized so the same
    builder is validated in CoreSim at mini scale."""
    _ensure_paths()
    import concourse.bass as bass
    import concourse.mybir as mybir
    import concourse.tile as tile
    from concourse.masks import make_identity
    from contextlib import ExitStack

    f32 = mybir.dt.float32
    bf16 = mybir.dt.bfloat16
    i16 = mybir.dt.int16
    Exp = mybir.ActivationFunctionType.Exp
    Log = mybir.ActivationFunctionType.Ln

    EC = e // 128      # E (contraction) chunks for projections
    TT = t // 512      # token tiles (projection streaming)
    TTQ = tq // 512    # token tiles that also need Q projection
    KC = t // 128      # key chunks (attention contraction)
    QTN = tq // 512    # query tiles in attention
    GK = gfd // 512    # key chunks per exp group
    NG = KC // GK      # exp groups per query tile
    assert KC % GK == 0 and GK == 2

    scale = 1.0 / float(np.sqrt(H))
    schrd_a = float(128.0 / np.log(2.0)) * scale
    schrd_b = float(127 * 128 - SCHRD_C + 0.5)

    nc = bass.Bass()
    # host-swizzled x: [n, p, c, u] so each token tile is one dense 2D DMA
    xT4 = nc.declare_dram_parameter("xT4", [TT, 128, EC, 512], bf16, False)
    wkv = nc.declare_dram_parameter("wkv", [e, 2 * H], bf16, False)
    wq = nc.declare_dram_parameter("wq", [e, H], bf16, False)
    outT = nc.declare_dram_parameter("outT", [H, tq], f32, True)

    wkv_r = wkv.rearrange("(c p) m -> p c m", p=128)
    wq_r = wq.rearrange("(c p) m -> p c m", p=128)

    with ExitStack() as ctx:
        tc = ctx.enter_context(tile.TileContext(nc))
        singles = ctx.enter_context(tc.tile_pool(name="singles", bufs=1))
        ppool = ctx.enter_context(tc.tile_pool(name="ppool", bufs=3))
        rpool = ctx.enter_context(tc.tile_pool(name="rpool", bufs=2))
        spool = ctx.enter_context(tc.tile_pool(name="spool", bufs=3, space="PSUM"))
        opool = ctx.enter_context(tc.tile_pool(name="opool", bufs=2, space="PSUM"))

        wkv_sb = singles.tile([128, EC, 2 * H], bf16)
        nc.sync.dma_start(out=wkv_sb, in_=wkv_r)
        wq_sb = singles.tile([128, EC, H], bf16)
        nc.sync.dma_start(out=wq_sb, in_=wq_r)

        # full x prefetch: one dense [128, EC*512] DMA per token tile; the
        # first two tiles go per-chunk so the first matmuls start early
        xt = [singles.tile([128, EC, 512], bf16, name=f"x{n}") for n in range(TT)]
        for n in range(TT):
            if n < 2:
                for c in range(EC):
                    nc.sync.dma_start(out=xt[n][:, c, :], in_=xT4[n][:, c, :])
            else:
                nc.sync.dma_start(out=xt[n], in_=xT4[n])

        # K^T rowtiled: [128, 256] per token tile -- even key chunks on
        # partitions 0:64, odd on 64:128, two 128-col blocks per tile.
        KTp = [singles.tile([128, 256], bf16, name=f"KT{n}") for n in range(TT)]
        # Q^T per query tile, duplicated across both partition halves (each
        # concurrent row-tile streams rhs from its own partition range).
        QTp = [singles.tile([128, 512], bf16, name=f"QT{n}") for n in range(TTQ)]
        # V^T for a PAIR of token tiles, partition-stacked [128, 512] so one
        # PE transpose handles two key chunks (one per tile of the pair)
        VTtp = [singles.tile([128, 512], bf16, name=f"VTt{n}") for n in range(TT // 2)]
        # PV stationary: [V_chunk | ones] per key chunk, ones replicated to
        # fill M=128 so rows 64:128 of the PV accumulator hold the softmax
        # denominator. One big tile; V written by DMA-xbar transpose.
        Vst3 = singles.tile([128, KC, 128], bf16)
        if use_gpsimd_memset:
            nc.gpsimd.memset(Vst3[:, :, H:], 1.0)
        else:
            nc.vector.memset(Vst3[:, :, H:], 1.0)
        OTp = [singles.tile([H, 512], f32, name=f"OT{q}") for q in range(QTN)]
        if not use_dma_transpose:
            ident = singles.tile([128, 128], bf16)
            make_identity(nc, ident)

        o_ps_list = [None] * QTN
        gctr = [0]  # global group counter for the exp-engine pattern
        next_g0 = [0]  # next q0 group to emit during the projection loop

        def emit_group(q, g, o_ps):
            s_ps = spool.tile([128, gfd], f32, tag="s", name=f"s{q}_{g}")
            kt = KTp[g // 2][:, (g % 2) * 128:(g % 2 + 1) * 128]
            nc.tensor.matmul(
                s_ps[:, 0:512], kt[0:H], QTp[q][0:H, :],
                start=True, stop=True, skip_group_check=True,
            )
            nc.tensor.matmul(
                s_ps[:, 512:1024], kt[H:128], QTp[q][H:128, :],
                start=True, stop=True, skip_group_check=True,
                tile_position=(64, 0),
            )
            pt = ppool.tile([128, gfd], bf16, tag="p", name=f"p{q}_{g}")
            if (gctr[0] % 8) in dve_slots:
                nc.vector.tensor_scalar(
                    out=pt[:].bitcast(i16),
                    in0=s_ps[:],
                    scalar1=schrd_a,
                    scalar2=schrd_b,
                    op0=mybir.AluOpType.mult,
                    op1=mybir.AluOpType.add,
                )
            else:
                nc.scalar.activation(pt, s_ps, Exp, scale=scale)
            gctr[0] += 1
            for k in range(GK):
                c = g * GK + k
                nc.tensor.matmul(
                    o_ps, Vst3[:, c, :], pt[:, k * 512:(k + 1) * 512],
                    start=(c == 0), stop=(c == KC - 1),
                    skip_group_check=True,
                )

        def emit_finalize(q, o_ps):
            rec = rpool.tile([H, 512], f32, tag="rec", name=f"rec{q}")
            if use_recip_approx:
                # 1/l = exp(-ln l) on ScalarE; ln+exp share one table set
                # (natural_log_exp_and_others) so no mid-kernel reload.
                lnl = rpool.tile([H, 512], f32, tag="lnl", name=f"lnl{q}")
                nc.scalar.activation(lnl, o_ps[H:128, :], Log)
                nc.scalar.activation(rec, lnl, Exp, scale=-1.0)
            else:
                nc.vector.reciprocal(rec, o_ps[H:128, :])
            nc.vector.tensor_mul(OTp[q][:], o_ps[0:H, :], rec)
            nc.sync.dma_start(
                out=outT[:, q * 512:(q + 1) * 512], in_=OTp[q][:]
            )

        # ---- emission: projections interleaved with q0 attention ----
        for n in range(TT):
            kv_ps = spool.tile([128, 512], f32, tag="s", name=f"kv{n}")
            for c in range(EC):
                nc.tensor.matmul(
                    kv_ps, wkv_sb[:, c, :], xt[n][:, c, :],
                    start=(c == 0), stop=(c == EC - 1),
                )
            srcv = kv_ps[0:H, :].rearrange("h (i r u) -> h i r u", i=2, r=2, u=128)
            dst = KTp[n].rearrange("p (i u) -> p i u", u=128)
            nc.vector.tensor_copy(dst[0:H], srcv[:, :, 0, :])
            nc.vector.tensor_copy(dst[H:128], srcv[:, :, 1, :])
            half = slice((n % 2) * H, (n % 2 + 1) * H)
            nc.vector.tensor_copy(VTtp[n // 2][half, :], kv_ps[H:128, :])
            if use_dma_transpose:
                for j in range(4):
                    c = 4 * n + j
                    nc.scalar.dma_start_transpose(
                        out=Vst3[:, c, 0:H],
                        in_=VTtp[n // 2][half, j * 128:(j + 1) * 128],
                    )
            elif n % 2 == 1:
                # one [128,128] PE transpose covers the same key chunk of
                # both tiles in the pair (V halves land side by side)
                for j in range(4):
                    tp = opool.tile([128, 128], bf16, tag="o", name=f"tp{n}_{j}")
                    nc.tensor.transpose(
                        tp, VTtp[n // 2][:, j * 128:(j + 1) * 128], ident
                    )
                    nc.vector.tensor_copy(Vst3[:, 4 * (n - 1) + j, 0:H], tp[:, 0:H])
                    nc.vector.tensor_copy(Vst3[:, 4 * n + j, 0:H], tp[:, H:128])
            if n < TTQ:
                q_ps = spool.tile([H, 512], f32, tag="s", name=f"q{n}")
                for c in range(EC):
                    nc.tensor.matmul(
                        q_ps, wq_sb[:, c, :], xt[n][:, c, :],
                        start=(c == 0), stop=(c == EC - 1),
                    )
                nc.vector.tensor_copy(QTp[n][0:H, :], q_ps)
                if use_scalar_dup:
                    nc.scalar.dma_start(out=QTp[n][H:128, :], in_=QTp[n][0:H, :])
                else:
                    nc.vector.tensor_copy(QTp[n][H:128, :], q_ps)
            # q0 attention groups feasible with the V chunks ready so far
            if o_ps_list[0] is None:
                o_ps_list[0] = opool.tile([128, 512], f32, tag="o", name="o0")
            ready = n + 1 if (use_dma_transpose or n % 2 == 1) else n
            gmax = min(2 * ready, NG)
            while next_g0[0] < gmax:
                emit_group(0, next_g0[0], o_ps_list[0])
                next_g0[0] += 1

        for g in range(next_g0[0], NG):
            emit_group(0, g, o_ps_list[0])
        emit_finalize(0, o_ps_list[0])
        for q in range(1, QTN):
            o_ps_list[q] = opool.tile([128, 512], f32, tag="o", name=f"o{q}")
            for g in range(NG):
                emit_group(q, g, o_ps_list[q])
            emit_finalize(q, o_ps_list[q])

    if legalize:
        _legalize_waits(nc, __import__("concourse.mybir", fromlist=["x"]))
    return nc


def _get_nc():
    key = (E, T, TQ)
    if key not in _BUILT:
        _BUILT[key] = build_nc()
    return _BUILT[key]


def _swizzle_xT(xT):
    """[E, T] f32 -> [TT, 128, EC, 512] bf16 tile-contiguous layout."""
    import ml_dtypes

    e, t = xT.shape
    ec, tt = e // 128, t // 512
    v = xT.reshape(ec, 128, tt, 512).transpose(2, 1, 0, 3)
    return np.ascontiguousarray(v).astype(ml_dtypes.bfloat16)


def kernel(x, Wq, Wk, Wv):
    """Full inputs -> full output, distributing over 8 NeuronCores."""
    _ensure_paths()
    _install_ntff_hook()
    import ml_dtypes
    from concourse.bass_utils import run_bass_kernel_spmd

    global LAST_RESULT

    nc = _get_nc()

    x = np.asarray(x, np.float32)
    wkv_np = np.ascontiguousarray(
        np.concatenate([np.asarray(Wk, np.float32), np.asarray(Wv, np.float32)], axis=1)
    ).astype(ml_dtypes.bfloat16)
    wq_np = np.ascontiguousarray(np.asarray(Wq, np.float32)).astype(ml_dtypes.bfloat16)

    in_maps = []
    for core in range(NCORES):
        b, half = divmod(core, 2)
        o = TQ if half == 0 else 0
        idx = np.r_[half * TQ:(half + 1) * TQ, o:o + TQ]
        xT4 = _swizzle_xT(np.ascontiguousarray(x[b, idx].T))
        in_maps.append({"xT4": xT4, "wkv": wkv_np, "wq": wq_np})

    trace = bool(os.environ.get("KERNEL_TRACE"))
    res = run_bass_kernel_spmd(nc, in_maps, list(range(NCORES)), trace=trace)
    LAST_RESULT = res

    out = np.empty((B, T, H), np.float32)
    for core in range(NCORES):
        b, half = divmod(core, 2)
        out[b, half * TQ:(half + 1) * TQ, :] = res.results[core]["outT"].T
    return out


# revision 20
# speedup vs baseline: 1.2827x; 1.0585x over previous
"""Single-head attention (B=4, T=4096, E=1024, H=64) on 8 trn2 NeuronCores.

Sharding: 2 cores per batch element; each core computes the full K/V
projections for its batch element but only its half of the queries
(sequence-parallel over queries, data-parallel over batch). The host
permutes each core's token order so its own query half comes first —
attention is permutation-invariant over keys, so every core runs an
identical SPMD program with no collectives.

Per-core on-chip pipeline (all layouts transposed, feature-on-partition):
  x is host-swizzled tile-contiguous and fully prefetched to SBUF in
  8 x 1MB dense DMAs (decouples DMA from compute pacing).
  xT [E,T] bf16 --matmul (Wk|Wv) packed--> K^T,V^T [64,T] f32
             --matmul Wq (first T/2 cols)--> Q^T [64,T/2]
  V^T --DMA-xbar transpose (Activation HWDGE queue)--> Vst [T/128,128,64|ones]
  S^T chunk = K^T_chunk.T @ Q^T  (contraction H=64, rowtiled, PSUM [128,1024])
  P^T = exp(S^T/8): split across two engines per a static pattern --
        ScalarE table exp, or VectorE Schraudolph exp (tensor_scalar
        f32*A+B -> int16 bits reinterpreted as bf16; ~1.8% rms)
  O^T += [V|ones].T @ P^T  (PSUM [128,512]: rows 0:64 = O^T,
                            rows 64:128 = softmax denominator)
  out = O^T * exp(-ln l)  (ScalarE recip, VectorE mul), DMA out [64, T/2] f32
"""

import os
import sys

import numpy as np

E, T, H, B = 1024, 4096, 64, 4
NCORES = 8
TQ = T // 2

_BUILT = {}
LAST_RESULT = None  # stashed BassKernelResults for test harness introspection

# Schraudolph bf16 exp: bits_i16 = floor(x*SCHRD_A + SCHRD_B); the +0.5 in B
# converts the observed floor conversion into round-half-up. C=7.5 centers
# the relative error (~1.8% rms, mean ~0).
SCHRD_C = 7.5
# number of groups (of 8) routed to the DVE Schraudolph exp; rest -> ScalarE
DVE_SLOTS = frozenset((2, 5, 7))


def _ensure_paths():
    for p in ("/opt/trn_rl_repo",):
        if p not in sys.path:
            sys.path.insert(0, p)


def _legalize_waits(nc, mybir, max_waits=1):
    """This walrus build only accepts 1 sem-wait per instruction; Tile's
    tail drains carry several. Move excess waits onto injected NoOps on
    the same engine right before the offending instruction."""
    ctr = 0
    for bb in nc.main_func.blocks:
        new_list = []
        for ins in bb.instructions:
            si = ins.sync_info
            if si is not None and len(si.on_wait) > max_waits:
                waits = list(si.on_wait)
                extra, keep = waits[:-max_waits], waits[-max_waits:]
                while extra:
                    chunk, extra = extra[:max_waits], extra[max_waits:]
                    ctr += 1
                    nop = mybir.InstNoOp(name=f"WFIX-{id(nc) & 0xFFFF}-{ctr}")
                    nop.engine = ins.engine
                    nop.sync_info = mybir.SyncInfo(on_wait=chunk, on_update=[])
                    new_list.append(nop)
                ins.sync_info = mybir.SyncInfo(
                    on_wait=keep, on_update=list(si.on_update)
                )
            new_list.append(ins)
        bb.instructions[:] = new_list


def _install_ntff_hook():
    """The image's antenv lacks axon_hooks, so trace=True degrades. Inject
    the module backed by the boot helper's ctypes implementation."""
    import types

    if "antenv.axon_hooks" in sys.modules:
        return
    if "/root/.axon_site" not in sys.path:
        sys.path.insert(0, "/root/.axon_site")
    try:
        from trn_agent_boot.trn_boot import _ntff_profile_via_ctypes

        hook = _ntff_profile_via_ctypes("/opt/axon/libaxon_pjrt.so")
    except Exception:
        return
    mod = types.ModuleType("antenv.axon_hooks")
    mod.get_axon_ntff_profile_hook = lambda: hook
    mod.set_axon_ntff_profile_hook = lambda h: None
    sys.modules["antenv.axon_hooks"] = mod


def build_nc(
    e=E,
    t=T,
    tq=TQ,
    gfd=1024,
    dve_slots=DVE_SLOTS,
    legalize=True,
    use_dma_transpose=False,
    use_scalar_dup=True,
    use_gpsimd_memset=True,
    use_recip_approx=True,
):
    """Emit the SPMD per-core program. Shapes parameterized so the same
    builder is validated in CoreSim at mini scale."""
    _ensure_paths()
    import concourse.bass as bass
    import concourse.mybir as mybir
    import concourse.tile as tile
    from concourse.masks import make_identity
    from contextlib import ExitStack

    f32 = mybir.dt.float32
    bf16 = mybir.dt.bfloat16
    i16 = mybir.dt.int16
    Exp = mybir.ActivationFunctionType.Exp
    Log = mybir.ActivationFunctionType.Ln

    EC = e // 128      # E (contraction) chunks for projections
    TT = t // 512      # token tiles (projection streaming)
    TTQ = tq // 512    # token tiles that also need Q projection
    KC = t // 128      # key chunks (attention contraction)
    QTN = tq // 512    # query tiles in attention
    GK = gfd // 512    # key chunks per exp group
    NG = KC // GK      # exp groups per query tile
    assert KC % GK == 0 and GK == 2

    scale = 1.0 / float(np.sqrt(H))
    schrd_a = float(128.0 / np.log(2.0)) * scale
    schrd_b = float(127 * 128 - SCHRD_C + 0.5)

    nc = bass.Bass()
    # host-swizzled x: [n, p, c, u] so each token tile is one dense 2D DMA
    xT4 = nc.declare_dram_parameter("xT4", [TT, 128, EC, 512], bf16, False)
    wkv = nc.declare_dram_parameter("wkv", [e, 2 * H], bf16, False)
    wq = nc.declare_dram_parameter("wq", [e, H], bf16, False)
    outT = nc.declare_dram_parameter("outT", [H, tq], f32, True)

    wkv_r = wkv.rearrange("(c p) m -> p c m", p=128)
    wq_r = wq.rearrange("(c p) m -> p c m", p=128)

    with ExitStack() as ctx:
        tc = ctx.enter_context(tile.TileContext(nc))
        singles = ctx.enter_context(tc.tile_pool(name="singles", bufs=1))
        ppool = ctx.enter_context(tc.tile_pool(name="ppool", bufs=3))
        rpool = ctx.enter_context(tc.tile_pool(name="rpool", bufs=2))
        spool = ctx.enter_context(tc.tile_pool(name="spool", bufs=3, space="PSUM"))
        opool = ctx.enter_context(tc.tile_pool(name="opool", bufs=2, space="PSUM"))

        wkv_sb = singles.tile([128, EC, 2 * H], bf16)
        nc.sync.dma_start(out=wkv_sb, in_=wkv_r)
        wq_sb = singles.tile([128, EC, H], bf16)
        nc.sync.dma_start(out=wq_sb, in_=wq_r)

        # full x prefetch: one dense [128, EC*512] DMA per token tile
        # (whole-tile 8KB-line descriptors run ~2.5x faster than 1KB lines)
        xt = [singles.tile([128, EC, 512], bf16, name=f"x{n}") for n in range(TT)]
        for n in range(TT):
            nc.sync.dma_start(out=xt[n], in_=xT4[n])

        # K^T rowtiled: [128, 256] per token tile -- even key chunks on
        # partitions 0:64, odd on 64:128, two 128-col blocks per tile.
        KTp = [singles.tile([128, 256], bf16, name=f"KT{n}") for n in range(TT)]
        # Q^T per query tile, duplicated across both partition halves (each
        # concurrent row-tile streams rhs from its own partition range).
        QTp = [singles.tile([128, 512], bf16, name=f"QT{n}") for n in range(TTQ)]
        # V^T for a PAIR of token tiles, partition-stacked [128, 512] so one
        # PE transpose handles two key chunks (one per tile of the pair)
        VTtp = [singles.tile([128, 512], bf16, name=f"VTt{n}") for n in range(TT // 2)]
        # PV stationary: [V_chunk | ones] per key chunk, ones replicated to
        # fill M=128 so rows 64:128 of the PV accumulator hold the softmax
        # denominator. One big tile; V written by DMA-xbar transpose.
        Vst3 = singles.tile([128, KC, 128], bf16)
        if use_gpsimd_memset:
            nc.gpsimd.memset(Vst3[:, :, H:], 1.0)
        else:
            nc.vector.memset(Vst3[:, :, H:], 1.0)
        OTp = [singles.tile([H, 512], f32, name=f"OT{q}") for q in range(QTN)]
        if not use_dma_transpose:
            ident = singles.tile([128, 128], bf16)
            make_identity(nc, ident)

        o_ps_list = [None] * QTN
        gctr = [0]  # global group counter for the exp-engine pattern
        next_g0 = [0]  # next q0 group to emit during the projection loop

        # Software pipelining: the PE queue is strictly in-order, so PV(g)
        # (which waits on exp(g)) must NOT directly follow QK(g) — emit
        # QK(g+1) between them so the PE streams matmuls while the exp
        # engines work. pv_backlog holds the one deferred PV stage.
        pv_backlog = []

        def emit_pv(q, g, pt, o_ps):
            for k in range(GK):
                c = g * GK + k
                nc.tensor.matmul(
                    o_ps, Vst3[:, c, :], pt[:, k * 512:(k + 1) * 512],
                    start=(c == 0), stop=(c == KC - 1),
                    skip_group_check=True,
                )
            if g == NG - 1:
                emit_finalize(q, o_ps)

        def emit_group(q, g, o_ps):
            s_ps = spool.tile([128, gfd], f32, tag="s", name=f"s{q}_{g}")
            kt = KTp[g // 2][:, (g % 2) * 128:(g % 2 + 1) * 128]
            nc.tensor.matmul(
                s_ps[:, 0:512], kt[0:H], QTp[q][0:H, :],
                start=True, stop=True, skip_group_check=True,
            )
            nc.tensor.matmul(
                s_ps[:, 512:1024], kt[H:128], QTp[q][H:128, :],
                start=True, stop=True, skip_group_check=True,
                tile_position=(64, 0),
            )
            pt = ppool.tile([128, gfd], bf16, tag="p", name=f"p{q}_{g}")
            if (gctr[0] % 8) in dve_slots:
                nc.vector.tensor_scalar(
                    out=pt[:].bitcast(i16),
                    in0=s_ps[:],
                    scalar1=schrd_a,
                    scalar2=schrd_b,
                    op0=mybir.AluOpType.mult,
                    op1=mybir.AluOpType.add,
                )
            else:
                nc.scalar.activation(pt, s_ps, Exp, scale=scale)
            gctr[0] += 1
            while pv_backlog:
                emit_pv(*pv_backlog.pop(0))
            pv_backlog.append((q, g, pt, o_ps))

        def flush_pv():
            while pv_backlog:
                emit_pv(*pv_backlog.pop(0))

        def emit_finalize(q, o_ps):
            rec = rpool.tile([H, 512], f32, tag="rec", name=f"rec{q}")
            if use_recip_approx:
                # 1/l = exp(-ln l) on ScalarE; ln+exp share one table set
                # (natural_log_exp_and_others) so no mid-kernel reload.
                lnl = rpool.tile([H, 512], f32, tag="lnl", name=f"lnl{q}")
                nc.scalar.activation(lnl, o_ps[H:128, :], Log)
                nc.scalar.activation(rec, lnl, Exp, scale=-1.0)
            else:
                nc.vector.reciprocal(rec, o_ps[H:128, :])
            nc.vector.tensor_mul(OTp[q][:], o_ps[0:H, :], rec)
            nc.sync.dma_start(
                out=outT[:, q * 512:(q + 1) * 512], in_=OTp[q][:]
            )

        # ---- emission: projections interleaved with q0 attention ----
        for n in range(TT):
            kv_ps = spool.tile([128, 512], f32, tag="s", name=f"kv{n}")
            for c in range(EC):
                nc.tensor.matmul(
                    kv_ps, wkv_sb[:, c, :], xt[n][:, c, :],
                    start=(c == 0), stop=(c == EC - 1),
                )
            srcv = kv_ps[0:H, :].rearrange("h (i r u) -> h i r u", i=2, r=2, u=128)
            dst = KTp[n].rearrange("p (i u) -> p i u", u=128)
            nc.vector.tensor_copy(dst[0:H], srcv[:, :, 0, :])
            nc.vector.tensor_copy(dst[H:128], srcv[:, :, 1, :])
            half = slice((n % 2) * H, (n % 2 + 1) * H)
            nc.vector.tensor_copy(VTtp[n // 2][half, :], kv_ps[H:128, :])
            if use_dma_transpose:
                for j in range(4):
                    c = 4 * n + j
                    nc.scalar.dma_start_transpose(
                        out=Vst3[:, c, 0:H],
                        in_=VTtp[n // 2][half, j * 128:(j + 1) * 128],
                    )
            elif n % 2 == 1:
                # one [128,128] PE transpose covers the same key chunk of
                # both tiles in the pair (V halves land side by side)
                for j in range(4):
                    tp = opool.tile([128, 128], bf16, tag="o", name=f"tp{n}_{j}")
                    nc.tensor.transpose(
                        tp, VTtp[n // 2][:, j * 128:(j + 1) * 128], ident
                    )
                    nc.vector.tensor_copy(Vst3[:, 4 * (n - 1) + j, 0:H], tp[:, 0:H])
                    nc.vector.tensor_copy(Vst3[:, 4 * n + j, 0:H], tp[:, H:128])
            if n < TTQ:
                q_ps = spool.tile([H, 512], f32, tag="s", name=f"q{n}")
                for c in range(EC):
                    nc.tensor.matmul(
                        q_ps, wq_sb[:, c, :], xt[n][:, c, :],
                        start=(c == 0), stop=(c == EC - 1),
                    )
                nc.vector.tensor_copy(QTp[n][0:H, :], q_ps)
                if use_scalar_dup:
                    nc.scalar.dma_start(out=QTp[n][H:128, :], in_=QTp[n][0:H, :])
                else:
                    nc.vector.tensor_copy(QTp[n][H:128, :], q_ps)
            # q0 attention groups feasible with the V chunks ready so far
            if o_ps_list[0] is None:
                o_ps_list[0] = opool.tile([128, 512], f32, tag="o", name="o0")
            ready = n + 1 if (use_dma_transpose or n % 2 == 1) else n
            gmax = min(2 * ready, NG)
            while next_g0[0] < gmax:
                emit_group(0, next_g0[0], o_ps_list[0])
                next_g0[0] += 1

        for g in range(next_g0[0], NG):
            emit_group(0, g, o_ps_list[0])
        for q in range(1, QTN):
            o_ps_list[q] = opool.tile([128, 512], f32, tag="o", name=f"o{q}")
            for g in range(NG):
                emit_group(q, g, o_ps_list[q])
        flush_pv()

    if legalize:
        _legalize_waits(nc, __import__("concourse.mybir", fromlist=["x"]))
    return nc


def _get_nc():
    key = (E, T, TQ)
    if key not in _BUILT:
        _BUILT[key] = build_nc()
    return _BUILT[key]


def _swizzle_xT(xT):
    """[E, T] f32 -> [TT, 128, EC, 512] bf16 tile-contiguous layout."""
    import ml_dtypes

    e, t = xT.shape
    ec, tt = e // 128, t // 512
    v = xT.reshape(ec, 128, tt, 512).transpose(2, 1, 0, 3)
    return np.ascontiguousarray(v).astype(ml_dtypes.bfloat16)


def kernel(x, Wq, Wk, Wv):
    """Full inputs -> full output, distributing over 8 NeuronCores."""
    _ensure_paths()
    _install_ntff_hook()
    import ml_dtypes
    from concourse.bass_utils import run_bass_kernel_spmd

    global LAST_RESULT

    nc = _get_nc()

    x = np.asarray(x, np.float32)
    wkv_np = np.ascontiguousarray(
        np.concatenate([np.asarray(Wk, np.float32), np.asarray(Wv, np.float32)], axis=1)
    ).astype(ml_dtypes.bfloat16)
    wq_np = np.ascontiguousarray(np.asarray(Wq, np.float32)).astype(ml_dtypes.bfloat16)

    in_maps = []
    for core in range(NCORES):
        b, half = divmod(core, 2)
        o = TQ if half == 0 else 0
        idx = np.r_[half * TQ:(half + 1) * TQ, o:o + TQ]
        xT4 = _swizzle_xT(np.ascontiguousarray(x[b, idx].T))
        in_maps.append({"xT4": xT4, "wkv": wkv_np, "wq": wq_np})

    trace = bool(os.environ.get("KERNEL_TRACE"))
    res = run_bass_kernel_spmd(nc, in_maps, list(range(NCORES)), trace=trace)
    LAST_RESULT = res

    out = np.empty((B, T, H), np.float32)
    for core in range(NCORES):
        b, half = divmod(core, 2)
        out[b, half * TQ:(half + 1) * TQ, :] = res.results[core]["outT"].T
    return out
